# revision 7
# baseline (speedup 1.0000x reference)
"""Corr1d cost-volume kernel for Trainium2 (8 NeuronCores).

corr[b, d, h, x] = sum_c fL[b,c,h,x] * fR[b,c,h,x-d]  for x >= d, else 0.
Shapes: fL, fR = (4, 64, 256, 512) fp32; out = (4, 48, 256, 512) fp32.

Sharding: data-parallel over (batch, h-half): core i handles b = i//2,
h rows [128*(i%2), 128*(i%2)+128).

Per-core pipeline, per hp (2 h rows, hi=0/1 on c-partition halves):
  - 8 banded matmuls (contract c=64): lhsT = fL[c, 128-x-block],
    rhs = zero-padded fR window (192 cols starting at x' = 128m-47)
    -> PSUM windows at 256-col alignment (no bank crossing)
  - DVE (hi=0) / ACT (hi=1) plain copy PSUM -> SBUF fp16 stage (no masks:
    the 48-col zero pad in fR makes the x<d region exactly 0)
  - one contiguous [128, 1536] fp16 DMA stage -> DRAM (full windows)
Host extracts the band: out[d, h, 128m+q] = win[.., q, hi, m, c=q+47-d]
via a take_along_axis gather, then casts fp32.
"""
import numpy as np
from contextlib import ExitStack

import concourse.bass as bass
import concourse.tile as tile
import concourse.bacc as bacc
import concourse.mybir as mybir
from concourse import bass_utils

B, C, H, W = 4, 64, 256, 512
D = 48
NCORES = 8
HH = H // 2           # h rows per core
NB = 8                # batches per core
NH = 16               # h rows per batch
NHH = 8               # hp iterations per batch (2 rows each)
PAD = 48              # left zero pad on fR rows
WP = W + PAD          # padded fR row width
WIN = 192             # window width
NWIN = 4              # x-blocks of 128 per row
HWIN = 112            # per-partition-half band span (64 + 48)
STG = 2 * NWIN * HWIN  # 896 stage cols

fp16 = mybir.dt.float16
fp32 = mybir.dt.float32


def _build_nc():
    nc = bacc.Bacc("TRN2", target_bir_lowering=False, debug=False,
                   num_devices=NCORES)
    fL_d = nc.dram_tensor("fLc", [NB, 128, NHH * W], fp16,
                          kind="ExternalInput").ap()
    fR_d = nc.dram_tensor("fRc", [NB, 128, NHH * WP], fp16,
                          kind="ExternalInput").ap()
    out_d = nc.dram_tensor("outc", [NB, NHH, 128, STG], fp16,
                           kind="ExternalOutput").ap()

    with tile.TileContext(nc) as tc, ExitStack() as ctx:
        in_pool = ctx.enter_context(tc.tile_pool(name="inp", bufs=2))
        stg_pool = ctx.enter_context(tc.tile_pool(name="stg", bufs=3))
        ps_pool = ctx.enter_context(tc.tile_pool(name="ps", bufs=4,
                                                 space="PSUM"))

        for ib in range(NB):
            fl = in_pool.tile([128, NHH * W], fp16, tag="fl")
            fr = in_pool.tile([128, NHH * WP], fp16, tag="fr")
            nc.sync.dma_start(fl[:], fL_d[ib])
            nc.sync.dma_start(fr[:], fR_d[ib])

            for hp in range(NHH):
                pss = []
                for hi in range(2):
                    ps = ps_pool.tile([128, 1024], fp32, tag="ps")
                    pb = 64 * hi
                    for m in range(NWIN):
                        n = WIN if m < 3 else 175  # m=3 stops at row end
                        nc.tensor.matmul(
                            ps[:, 256 * m : 256 * m + n],
                            fl[pb : pb + 64,
                               hp * W + 128 * m : hp * W + 128 * m + 128],
                            fr[pb : pb + 64,
                               hp * WP + 128 * m + 1 :
                               hp * WP + 128 * m + 1 + n],
                            start=True, stop=True,
                        )
                    pss.append(ps)

                # each partition-half q = 64s+u only needs window cols
                # c in [64s, 64s+112): copy the needed half-window only
                stage = stg_pool.tile([128, STG], fp16)
                for hi, eng_copy in ((0, nc.vector.tensor_copy),
                                     (1, nc.scalar.copy)):
                    for s in range(2):
                        eng_copy(
                            stage[64 * s : 64 * s + 64,
                                  hi * NWIN * HWIN : (hi + 1) * NWIN * HWIN]
                            .rearrange("p (m c) -> p m c", m=NWIN),
                            pss[hi][64 * s : 64 * s + 64, :]
                            .rearrange("p (m c) -> p m c", m=NWIN)
                            [:, :, 64 * s : 64 * s + HWIN],
                        )
                eng = nc.sync if hp % 2 == 0 else nc.scalar
                eng.dma_start(out_d[ib, hp], stage[:])

    nc.compile()
    return nc


_NC_CACHE = None


def _get_nc():
    global _NC_CACHE
    if _NC_CACHE is None:
        _NC_CACHE = _build_nc()
    return _NC_CACHE


def _prep_core_inputs(fL, fR, core):
    b, half = divmod(core, 2)
    sl = np.s_[b, :, half * HH : half * HH + HH, :]
    fLs = fL[sl].astype(np.float16)                     # (64, 128, 512)
    fRs = fR[sl].astype(np.float16)
    frp = np.zeros((C, HH, WP), dtype=np.float16)
    frp[:, :, PAD:] = fRs
    # h = 16*ib + 8*hi + hp  ->  reshape h as (ib, hi, hp)
    fLc = (fLs.reshape(C, NB, 2, NHH, W)
           .transpose(1, 2, 0, 3, 4).reshape(NB, 128, NHH * W))
    fRc = (frp.reshape(C, NB, 2, NHH, WP)
           .transpose(1, 2, 0, 3, 4).reshape(NB, 128, NHH * WP))
    return np.ascontiguousarray(fLc), np.ascontiguousarray(fRc)


# stage col index for band extraction within a partition half:
# q = 64s+u, window col c = q+47-d, stored at cc = c - 64s = u + 47 - d
_CIDX = (np.arange(64)[:, None] + (D - 1) - np.arange(D)[None, :])  # (64,48)


def kernel(fL: np.ndarray, fR: np.ndarray) -> np.ndarray:
    fL = np.asarray(fL, dtype=np.float32)
    fR = np.asarray(fR, dtype=np.float32)
    nc = _get_nc()

    in_maps = []
    for core in range(NCORES):
        fLc, fRc = _prep_core_inputs(fL, fR, core)
        in_maps.append({"fLc": fLc, "fRc": fRc})

    res = bass_utils.run_bass_kernel_spmd(nc, in_maps,
                                          core_ids=list(range(NCORES)))
    out = np.empty((B, D, H, W), dtype=np.float32)
    cidx = _CIDX[None, None, None, None, None]          # (1,1,1,1,1,64,48)
    for core in range(NCORES):
        b, half = divmod(core, 2)
        win = res.results[core]["outc"].reshape(
            NB, NHH, 2, 64, 2, NWIN, HWIN)              # (ib,hp,s,u,hi,m,cc)
        wt = win.transpose(0, 4, 1, 5, 2, 3, 6)         # (ib,hi,hp,m,s,u,cc)
        band = np.take_along_axis(wt, cidx, axis=-1)    # (ib,hi,hp,m,s,u,48)
        # -> (d, ib, hi, hp, m, s, u) -> (48, 128, 512), x = 128m + 64s + u
        oc = band.transpose(6, 0, 1, 2, 3, 4, 5).reshape(D, HH, W)
        out[b, :, half * HH : half * HH + HH, :] = oc.astype(np.float32)
    return out


# revision 15
# speedup vs baseline: 1.2319x; 1.2319x over previous
"""Corr1d cost-volume kernel for Trainium2 (8 NeuronCores).

corr[b, d, h, x] = sum_c fL[b,c,h,x] * fR[b,c,h,x-d]  for x >= d, else 0.
Shapes: fL, fR = (4, 64, 256, 512) fp32; out = (4, 48, 256, 512) fp32.

Sharding: data-parallel over (batch, h-half): core i handles b = i//2,
h rows [128*(i%2), 128*(i%2)+128).

Per-core pipeline, per hp (2 h rows, hi=0/1 on c-partition halves):
  - 8 banded matmuls (contract c=64): lhsT = fL[c, 128-x-block],
    rhs = zero-padded fR window (192 cols starting at x' = 128m-47)
    -> PSUM windows at 256-col alignment (no bank crossing)
  - DVE (hi=0) / ACT (hi=1) plain copy PSUM -> SBUF fp16 stage (no masks:
    the 48-col zero pad in fR makes the x<d region exactly 0)
  - one contiguous [128, 1536] fp16 DMA stage -> DRAM (full windows)
Host extracts the band: out[d, h, 128m+q] = win[.., q, hi, m, c=q+47-d]
via a take_along_axis gather, then casts fp32.
"""
import numpy as np
from contextlib import ExitStack

import concourse.bass as bass
import concourse.tile as tile
import concourse.bacc as bacc
import concourse.mybir as mybir
from concourse import bass_utils

B, C, H, W = 4, 64, 256, 512
D = 48
NCORES = 8
HH = H // 2           # h rows per core
NB = 8                # batches per core
NH = 16               # h rows per batch
NHH = 8               # hp iterations per batch (2 rows each)
PAD = 48              # left zero pad on fR rows
WP = W + PAD          # padded fR row width
WIN = 192             # window width
NWIN = 4              # x-blocks of 128 per row
STG = 2 * NWIN * WIN  # 1536 stage cols

fp16 = mybir.dt.float16
fp32 = mybir.dt.float32


def _build_nc():
    nc = bacc.Bacc("TRN2", target_bir_lowering=False, debug=False,
                   num_devices=NCORES)
    fL_d = nc.dram_tensor("fLc", [NB, 128, NHH * W], fp16,
                          kind="ExternalInput").ap()
    fR_d = nc.dram_tensor("fRc", [NB, 128, NHH * WP], fp16,
                          kind="ExternalInput").ap()
    out_d = nc.dram_tensor("outc", [NB, NHH, 128, STG], fp16,
                           kind="ExternalOutput").ap()

    with tile.TileContext(nc) as tc, ExitStack() as ctx:
        in_pool = ctx.enter_context(tc.tile_pool(name="inp", bufs=2))
        stg_pool = ctx.enter_context(tc.tile_pool(name="stg", bufs=4))
        ps_pool = ctx.enter_context(tc.tile_pool(name="ps", bufs=4,
                                                 space="PSUM"))

        for ib in range(NB):
            fl = in_pool.tile([128, NHH * W], fp16, tag="fl")
            fr = in_pool.tile([128, NHH * WP], fp16, tag="fr")
            nc.sync.dma_start(fl[:], fL_d[ib])
            # fR load on the ACT HWDGE queue: balances the two DMA queues
            # (SP was carrying all inputs + half the outputs)
            nc.scalar.dma_start(fr[:], fR_d[ib])

            for hp in range(NHH):
                pss = []
                for hi in range(2):
                    ps = ps_pool.tile([128, 1024], fp32, tag="ps")
                    pb = 64 * hi
                    for m in range(NWIN):
                        n = WIN if m < 3 else 175  # m=3 stops at row end
                        nc.tensor.matmul(
                            ps[:, 256 * m : 256 * m + n],
                            fl[pb : pb + 64,
                               hp * W + 128 * m : hp * W + 128 * m + 128],
                            fr[pb : pb + 64,
                               hp * WP + 128 * m + 1 :
                               hp * WP + 128 * m + 1 + n],
                            start=True, stop=True,
                        )
                    pss.append(ps)

                stage = stg_pool.tile([128, STG], fp16)
                nc.vector.tensor_copy(
                    stage[:, 0 : NWIN * WIN]
                    .rearrange("p (m c) -> p m c", m=NWIN),
                    pss[0][:].rearrange("p (m c) -> p m c", m=NWIN)
                    [:, :, 0:WIN],
                )
                nc.scalar.copy(
                    stage[:, NWIN * WIN : STG]
                    .rearrange("p (m c) -> p m c", m=NWIN),
                    pss[1][:].rearrange("p (m c) -> p m c", m=NWIN)
                    [:, :, 0:WIN],
                )
                eng = nc.sync if hp % 2 == 0 else nc.scalar
                eng.dma_start(out_d[ib, hp], stage[:])

    nc.compile()
    return nc


_NC_CACHE = None


def _get_nc():
    global _NC_CACHE
    if _NC_CACHE is None:
        _NC_CACHE = _build_nc()
    return _NC_CACHE


def _prep_core_inputs(fL, fR, core):
    b, half = divmod(core, 2)
    sl = np.s_[b, :, half * HH : half * HH + HH, :]
    fLs = fL[sl].astype(np.float16)                     # (64, 128, 512)
    fRs = fR[sl].astype(np.float16)
    frp = np.zeros((C, HH, WP), dtype=np.float16)
    frp[:, :, PAD:] = fRs
    # h = 16*ib + 8*hi + hp  ->  reshape h as (ib, hi, hp)
    fLc = (fLs.reshape(C, NB, 2, NHH, W)
           .transpose(1, 2, 0, 3, 4).reshape(NB, 128, NHH * W))
    fRc = (frp.reshape(C, NB, 2, NHH, WP)
           .transpose(1, 2, 0, 3, 4).reshape(NB, 128, NHH * WP))
    return np.ascontiguousarray(fLc), np.ascontiguousarray(fRc)


# c index for band extraction: c = q + 47 - d   (q: x within 128-block)
_CIDX = (np.arange(128)[:, None] + (D - 1) - np.arange(D)[None, :])  # (128,48)


def kernel(fL: np.ndarray, fR: np.ndarray) -> np.ndarray:
    fL = np.asarray(fL, dtype=np.float32)
    fR = np.asarray(fR, dtype=np.float32)
    nc = _get_nc()

    in_maps = []
    for core in range(NCORES):
        fLc, fRc = _prep_core_inputs(fL, fR, core)
        in_maps.append({"fLc": fLc, "fRc": fRc})

    res = bass_utils.run_bass_kernel_spmd(nc, in_maps,
                                          core_ids=list(range(NCORES)))
    out = np.empty((B, D, H, W), dtype=np.float32)
    cidx = _CIDX[None, None, None, None]                # (1,1,1,1,128,48)
    for core in range(NCORES):
        b, half = divmod(core, 2)
        win = res.results[core]["outc"].reshape(NB, NHH, 128, 2, NWIN, WIN)
        # -> (ib, hi, hp, m, q, c)
        wt = win.transpose(0, 3, 1, 4, 2, 5)
        band = np.take_along_axis(wt, cidx, axis=-1)    # (ib,hi,hp,m,q,48)
        # -> (d, ib, hi, hp, m, q) -> (48, 128, 512)
        oc = band.transpose(5, 0, 1, 2, 3, 4).reshape(D, HH, W)
        out[b, :, half * HH : half * HH + HH, :] = oc.astype(np.float32)
    return out
